# revision 36
# baseline (speedup 1.0000x reference)
"""LightGCN message-passing + BPR loss on 8 Trainium2 NeuronCores.

Dest-sharded SpMM with pipelined AllGathers. Nodes are permuted into 8 core
shards (dealt round-robin by degree), each shard split into TWO row regions
(72 + 75 dest-windows of 128 rows). The inter-layer exchange is TWO
AllGather collectives (one per region) so each layer's compute overlaps the
other region's collective: every layer runs as region-phases — phase p of
any dest-window only gathers source embeddings from source-region p, so
phase p of layer l+1 depends only on AG chunk p of layer l.

Per 128-token chunk a selection matrix S[token, dest_local] =
val(token) * (dlocal(token) == dest_local) is built with one DVE
tensor_scalar op and matmul-accumulated into the dest window's PSUM tile
(one PSUM accumulation group per phase; phases summed via an SBUF ysum
tile). The BPR head computes the L2-reg term fully locally (each sampled
row is owned by exactly one core) followed by a tiny scalar AllReduce that
runs under the layer compute; only the pooled-embedding compact+AllGather
remains on the critical tail.

Returns (loss1, reg_loss) like the reference.
"""
import sys

sys.path.insert(0, "/opt/trn_rl_repo")

import numpy as np
import ml_dtypes

import concourse.bass as bass
import concourse.bacc as bacc
import concourse.tile as tile
from concourse import mybir, library_config

# ---------------- problem constants (hardcoded per spec) ----------------
NUM_USERS = 100000
NUM_ITEMS = 50000
DIM = 64
BATCH = 8192
NCORES = 8

P = 128                      # partitions / rows per dest window
NDW = 147                    # dest windows per core
SHARD = NDW * P              # 18816 rows per core
REG_DWS = (72, 75)           # dest windows per region
DW0 = (0, 72)                # first dw of each region
REG_ROWS = (REG_DWS[0] * P, REG_DWS[1] * P)            # 9216, 9600
REG_BASE = (0, REG_ROWS[0])                            # local row base
REG_GROWS = (NCORES * REG_ROWS[0], NCORES * REG_ROWS[1])  # 73728, 76800
NTOT_G = REG_GROWS[0] + REG_GROWS[1]                   # 150528
# source windows: 3 per region, int16-addressable
WIN_SIZE = (24576, 24576, 24576, 25600, 25600, 25600)
WIN_REG = (0, 0, 0, 1, 1, 1)
WIN_LBASE = (0, 24576, 49152, 0, 25600, 51200)         # base within region
LOCAL_W = 6                  # pseudo-window: source row owned by this core
NSW = 7
# phases per dw-chunk: phase 0 = local sources (no AG dependency),
# phase 1 = remote region-0 windows, phase 2 = remote region-1 windows
PHASES = ((LOCAL_W,), (0, 1, 2), (3, 4, 5))
# block emission order: both local phases first (they read shard_bounce,
# which ph2 blocks overwrite), then dwc0 remote (-> AG chunk 0), then dwc1
BLK_ORDER = ((0, 0), (1, 0), (0, 1), (0, 2), (1, 1), (1, 2))
SG = 8                       # dest windows per supergroup (PSUM banks)
MAXCH = 16                   # chunks (128 tokens each) per dma_gather

SG_BLOCKS = (
    [list(range(s, s + SG)) for s in range(0, REG_DWS[0], SG)],
    [list(range(s, min(s + SG, NDW))) for s in range(DW0[1], NDW, SG)],
)


# ---------------- host-side graph preprocessing ----------------
def _preprocess(edge_row, edge_col, edge_vals):
    n_nodes = NUM_USERS + NUM_ITEMS
    deg = np.bincount(edge_row, minlength=n_nodes)

    # Deal degree-sorted nodes round-robin over the 1176 (core, dw) slots in
    # snake order; partition lane = deal round.
    order = np.argsort(-deg, kind="stable")
    nslots = NCORES * NDW
    idx = np.arange(n_nodes)
    rounds = idx // nslots
    within = idx % nslots
    snake = np.where(rounds % 2 == 0, within, nslots - 1 - within)
    slot_ids = np.empty(n_nodes, dtype=np.int64)
    lane = np.empty(n_nodes, dtype=np.int64)
    slot_ids[order] = snake
    lane[order] = rounds
    core = slot_ids // NDW
    dw = slot_ids % NDW
    reg = (dw >= REG_DWS[0]).astype(np.int64)
    loc = np.where(reg == 0, dw * P, REG_BASE[1] + (dw - DW0[1]) * P) + lane
    gpos = np.where(reg == 0,
                    core * REG_ROWS[0] + dw * P,
                    REG_GROWS[0] + core * REG_ROWS[1] + (dw - DW0[1]) * P) + lane

    # token fields
    d_core = core[edge_row]
    d_dw = dw[edge_row]
    d_part = lane[edge_row]
    s_g = gpos[edge_col]
    s_r1 = s_g >= REG_GROWS[0]
    rem_sw = np.where(s_r1, 3 + (s_g - REG_GROWS[0]) // 25600, s_g // 24576)
    rem_sloc = np.where(s_r1, (s_g - REG_GROWS[0]) % 25600, s_g % 24576)
    is_local = core[edge_col] == d_core
    t_sw = np.where(is_local, LOCAL_W, rem_sw)
    t_sloc = np.where(is_local, loc[edge_col], rem_sloc)
    t_val = np.asarray(edge_vals, dtype=np.float32)

    # group rank order = block order, block = (dwc, phase, sg)
    NGR = NDW * NSW
    g_local = d_dw * NSW + t_sw
    ranked_groups = []
    for dwc, phi in BLK_ORDER:
        for sgdws in SG_BLOCKS[dwc]:
            for w in PHASES[phi]:
                for dwi in sgdws:
                    ranked_groups.append(dwi * NSW + w)
    ranked_groups = np.asarray(ranked_groups)
    rank_of_g = np.empty(NGR, dtype=np.int64)
    rank_of_g[ranked_groups] = np.arange(NGR)

    # counts per (core, group); shared schedule = max over cores, padded to
    # 16 tokens per group (idx-layout granularity); each (block, window)
    # segment padded to 128 so gather runs stay column-aligned
    cnt = np.zeros((NCORES, NGR), dtype=np.int64)
    np.add.at(cnt, (d_core, g_local), 1)
    n16_g = -(-cnt.max(axis=0) // 32) * 32         # [NGR] natural order
    grp_off = np.zeros(NGR, dtype=np.int64)
    seg_layout = {}                                # (blk_idx, w) -> (off, ncols)
    cur = 0
    bi = 0
    blk_bounds = []                                # (off, ntok) per block
    for dwc, phi in BLK_ORDER:
        for sgdws in SG_BLOCKS[dwc]:
            blk_off = cur
            for w in PHASES[phi]:
                seg_off = cur
                for dwi in sgdws:
                    g = dwi * NSW + w
                    grp_off[g] = cur
                    cur += int(n16_g[g])
                    if cur % P == 96:
                        # PE matmul base partition must be 0/32/64 — never
                        # start a group at row 96 of a column
                        cur += 32
                cur = -(-cur // P) * P
                seg_layout[(bi, w)] = (seg_off, (cur - seg_off) // P)
            blk_bounds.append((blk_off, cur - blk_off))
            bi += 1
    toktot = int(cur)

    # within-group index per token (stable order)
    K = d_core * NGR + rank_of_g[g_local]
    perm = np.argsort(K, kind="stable")
    Ks = K[perm]
    starts = np.r_[0, np.flatnonzero(np.diff(Ks)) + 1]
    grp_start = starts[np.searchsorted(Ks[starts], Ks)]
    within_grp = np.arange(len(Ks)) - grp_start
    tgt = grp_off[g_local[perm]] + within_grp
    c_perm = d_core[perm]

    sloc_arr = np.zeros((NCORES, toktot), dtype=np.int16)
    val_arr = np.zeros((NCORES, toktot), dtype=np.float32)
    dloc_arr = np.zeros((NCORES, toktot), dtype=np.float32)
    sloc_arr[c_perm, tgt] = t_sloc[perm].astype(np.int16)
    val_arr[c_perm, tgt] = t_val[perm]
    dloc_arr[c_perm, tgt] = d_part[perm].astype(np.float32)

    # schedule: blocks in processing order; each block = one (dwc, phi, sg).
    # Pieces are (dw, col, r0, r1) row-slices of 128-token columns — a
    # column can host (parts of) several 16-granular groups.
    sched = []
    bi = 0
    for dwc, phi in BLK_ORDER:
        for sgdws in SG_BLOCKS[dwc]:
            blk_off, blk_ntok = blk_bounds[bi]
            npieces = {d: 0 for d in sgdws}
            segs_raw = []
            for w in PHASES[phi]:
                seg_off, seg_ncols = seg_layout[(bi, w)]
                pieces = []
                for dwi in sgdws:
                    g = dwi * NSW + w
                    o = int(grp_off[g])
                    rem = int(n16_g[g])
                    while rem > 0:
                        r0 = o % P
                        take = min(P - r0, rem)
                        # split to PE-quadrant-legal (base, size) sub-pieces:
                        # base 0 any size, base 64 up to 64, base 32/96 up
                        # to 32
                        a, b = r0, r0 + take
                        while a < b:
                            if a == 0:
                                c = b
                            elif a == 32:
                                c = min(b, 64)
                            else:
                                assert a == 64, a
                                c = b
                            pieces.append([dwi, o // P, a, c])
                            npieces[dwi] += 1
                            a = c
                        o += take
                        rem -= take
                pieces.sort(key=lambda x: (x[1], x[2]))
                segs_raw.append((w, seg_off, seg_ncols, pieces))
            seen = {d: 0 for d in sgdws}
            segs = []
            for w, seg_off, seg_ncols, pieces in segs_raw:
                out = []
                for dwi, col, r0, r1 in pieces:
                    seen[dwi] += 1
                    out.append((dwi, col, r0, r1, seen[dwi] == 1,
                                seen[dwi] == npieces[dwi]))
                segs.append((w, seg_off, seg_ncols, out))
            sched.append(dict(dwc=dwc, phi=phi, dws=sgdws, off=blk_off,
                              ntok=blk_ntok, segs=segs))
            bi += 1

    return dict(core=core, loc=loc, sloc=sloc_arr, val=val_arr, dloc=dloc_arr,
                toktot=toktot, sched=sched)


def _idx16_layout(sloc_row):
    """int16 token array -> dma_gather idxs layout [128, n/16] (8x replicated)."""
    n = sloc_row.shape[0]
    a = np.zeros((16, n // 16), np.int16)
    a[np.arange(n) % 16, np.arange(n) // 16] = sloc_row
    return np.tile(a, (8, 1))


def _pm_layout(arr_row):
    """token array -> [128, n/128] (token t at [t%128, t//128])."""
    n = arr_row.shape[0]
    a = np.zeros((P, n // P), arr_row.dtype)
    a[np.arange(n) % P, np.arange(n) // P] = arr_row
    return a


# ---------------- device kernel ----------------
def _build_kernel(num_layers, sched, toktot, S3A, S3B, debug_outputs=False):
    nc = bacc.Bacc(None, target_bir_lowering=False, num_swdge_queues=4)
    f32 = mybir.dt.float32
    NB = BATCH // P
    max_blktok = max(b["ntok"] for b in sched)
    rg = [list(range(NCORES))]

    x0_shard = nc.dram_tensor("x0_shard", [SHARD, DIM], f32, kind="ExternalInput")
    tok_idx = nc.dram_tensor("tok_idx", [P, toktot // 16], mybir.dt.int16,
                             kind="ExternalInput")
    tok_val = nc.dram_tensor("tok_val", [P, toktot // P], f32, kind="ExternalInput")
    tok_dloc = nc.dram_tensor("tok_dloc", [P, toktot // P], f32, kind="ExternalInput")
    iota_in = nc.dram_tensor("iota_in", [P, P], mybir.dt.bfloat16,
                             kind="ExternalInput")
    bpr_u = nc.dram_tensor("bpr_u", [P, BATCH // 16], mybir.dt.int16, kind="ExternalInput")
    bpr_p = nc.dram_tensor("bpr_p", [P, BATCH // 16], mybir.dt.int16, kind="ExternalInput")
    bpr_n = nc.dram_tensor("bpr_n", [P, BATCH // 16], mybir.dt.int16, kind="ExternalInput")
    comp_idx = nc.dram_tensor("comp_idx", [P, (S3A + S3B) // 16],
                              mybir.dt.int16, kind="ExternalInput")
    reg_mult = nc.dram_tensor("reg_mult", [P, NDW], mybir.dt.float32,
                              kind="ExternalInput")
    reg_slots = nc.dram_tensor("reg_slots", [P, P // 16], mybir.dt.int16,
                               kind="ExternalInput")
    out_loss = nc.dram_tensor("out_loss", [1, 2], f32, kind="ExternalOutput")
    dbg = {}
    if debug_outputs:
        dbg["pooled_shard"] = nc.dram_tensor("pooled_shard_out", [SHARD, DIM],
                                             f32, kind="ExternalOutput")

    with tile.TileContext(nc) as tc:
        with (
            tc.tile_pool(name="persist", bufs=1) as pp,
            tc.tile_pool(name="idxs", bufs=2) as ipool,
            tc.tile_pool(name="gath", bufs=6) as gpool,
            tc.tile_pool(name="work", bufs=3) as wpool,
            tc.tile_pool(name="ys", bufs=1) as ypool,
            tc.tile_pool(name="bpr", bufs=1) as bpool,
            tc.tile_pool(name="psum", bufs=1, space="PSUM") as psum_pool,
            tc.tile_pool(name="dram", bufs=1, space="DRAM") as dram,
        ):
            with tc.tile_critical():
                nc.gpsimd.load_library(library_config.mlp)

            # per-layer tables, one DRAM tensor per region so Tile tracks
            # the two AG chunks as independent dependencies
            bf16 = mybir.dt.bfloat16
            tables = []
            for l in range(num_layers + 1):
                t0 = dram.tile([REG_GROWS[0], 2 * DIM], bf16, tag=f"table{l}r0")
                t1 = dram.tile([REG_GROWS[1], 2 * DIM], bf16, tag=f"table{l}r1")
                tables.append((t0, t1))
            shard_bounce = dram.tile([SHARD, 2 * DIM], bf16)
            pooled_bounce = dram.tile([SHARD, DIM], f32)
            comp_bounce = dram.tile([S3A + S3B + 1, DIM], f32)
            comp_table = dram.tile(
                [NCORES * S3A + NCORES * (S3B + 1), DIM], f32)

            iota = pp.tile([P, P], mybir.dt.bfloat16)
            nc.sync.dma_start(out=iota[:], in_=iota_in[:])

            accum = pp.tile([P, NDW * DIM], f32)
            nc.sync.dma_start(
                out=accum[:].rearrange("p (dw j) -> p dw j", j=DIM),
                in_=x0_shard[:].rearrange("(dw p) j -> p dw j", p=P))

            # x0 -> bounce -> per-region AllGather into table0
            for dwc in (0, 1):
                sl = slice(REG_BASE[dwc], REG_BASE[dwc] + REG_ROWS[dwc])
                nc.gpsimd.dma_start(out=shard_bounce[sl, 0:DIM],
                                    in_=x0_shard[sl, :])
                nc.gpsimd.collective_compute(
                    "AllGather", mybir.AluOpType.bypass, replica_groups=rg,
                    ins=[shard_bounce[sl, :].opt()],
                    outs=[tables[0][dwc][:].opt()])

            tok_val_t = pp.tile([P, toktot // P], f32)
            nc.sync.dma_start(out=tok_val_t[:], in_=tok_val[:])
            tok_dloc_t = pp.tile([P, toktot // P], f32)
            nc.sync.dma_start(out=tok_dloc_t[:], in_=tok_dloc[:])

            ones = pp.tile([P, 1], f32)
            nc.gpsimd.memset(ones[:], 1.0)

            # ---- L2-reg partial: multiplicity-weighted local square sum ----
            # reg = sum over sampled rows of |x0|^2 = sum_r mult(r) * |x0_r|^2.
            # Every sampled row is owned by exactly one core; accum == x0 at
            # this point. Partial travels as an extra row of the comp table's
            # AllGather (no extra collective, no gathers).
            reg_mult_t = pp.tile([P, NDW], f32)
            nc.sync.dma_start(out=reg_mult_t[:], in_=reg_mult[:])
            sq = ypool.tile([P, REG_DWS[1] * DIM], f32, tag="ysum1", name="sq")
            rr = pp.tile([P, 1], f32)
            base = 0
            for k, nd in enumerate((74, 73)):
                c0, c1 = base * DIM, (base + nd) * DIM
                nc.vector.tensor_tensor(out=sq[:, :nd * DIM],
                                        in0=accum[:, c0:c1],
                                        in1=accum[:, c0:c1],
                                        op=mybir.AluOpType.mult)
                r1 = pp.tile([P, 74], f32, tag="regr1", name="r1")
                nc.vector.tensor_reduce(
                    out=r1[:, :nd],
                    in_=sq[:, :nd * DIM].rearrange("p (d j) -> p d j", j=DIM),
                    axis=mybir.AxisListType.X, op=mybir.AluOpType.add)
                nc.vector.tensor_tensor(out=r1[:, :nd], in0=r1[:, :nd],
                                        in1=reg_mult_t[:, base:base + nd],
                                        op=mybir.AluOpType.mult)
                r2 = pp.tile([P, 1], f32, tag="regr2", name="r2")
                nc.vector.tensor_reduce(out=r2[:], in_=r1[:, :nd],
                                        axis=mybir.AxisListType.X,
                                        op=mybir.AluOpType.add)
                if k == 0:
                    nc.vector.tensor_copy(out=rr[:], in_=r2[:])
                else:
                    nc.vector.tensor_tensor(out=rr[:], in0=rr[:], in1=r2[:],
                                            op=mybir.AluOpType.add)
                base += nd
            reg_ps = psum_pool.tile([1, 1], f32, tag="ps0")
            nc.tensor.matmul(reg_ps[:], ones[:], rr[:], start=True, stop=True)
            regrow = pp.tile([1, DIM], f32)
            nc.gpsimd.memset(regrow[:], 0.0)
            nc.scalar.copy(out=regrow[:, 0:1], in_=reg_ps[:])

            # ---- layers ----
            for layer in range(1, num_layers + 1):
                src = tables[layer - 1]
                ysums = {}
                last_blk_of_dwc = {dwc: max(i for i, b in enumerate(sched)
                                            if b["dwc"] == dwc)
                                   for dwc in (0, 1)}
                for bi, blk in enumerate(sched):
                    dwc = blk["dwc"]
                    phi = blk["phi"]
                    sgdws = blk["dws"]
                    if dwc not in ysums:
                        ysums[dwc] = ypool.tile(
                            [P, REG_DWS[dwc] * DIM], f32, tag=f"ysum{dwc}",
                            name=f"ysum{dwc}")
                    ysum = ysums[dwc]
                    sg_idx = ipool.tile([P, max_blktok // 16],
                                        mybir.dt.int16, tag="sgidx")
                    nc.sync.dma_start(
                        out=sg_idx[:, : blk["ntok"] // 16],
                        in_=tok_idx[:, blk["off"] // 16:
                                    (blk["off"] + blk["ntok"]) // 16])
                    ptiles = {}
                    for w, seg_off, seg_ncols, pieces in blk["segs"]:
                        if w == LOCAL_W:
                            src_win = shard_bounce[:, :]
                        else:
                            src_win = src[WIN_REG[w]][
                                WIN_LBASE[w]:WIN_LBASE[w] + WIN_SIZE[w], :]
                        seg_col0 = seg_off // P
                        pi_ = 0
                        for rc0 in range(0, seg_ncols, MAXCH):
                            ncols = min(MAXCH, seg_ncols - rc0)
                            ntok = ncols * P
                            g = gpool.tile([P, MAXCH, 2 * DIM],
                                           mybir.dt.bfloat16, tag="g")
                            locw = (seg_off + rc0 * P - blk["off"]) // 16
                            nc.gpsimd.dma_gather(
                                g[:, :ncols, :], src_win,
                                sg_idx[:, locw:locw + ntok // 16],
                                ntok, ntok, 2 * DIM, single_packet=False)
                            run_end = seg_col0 + rc0 + ncols
                            while pi_ < len(pieces) and pieces[pi_][1] < run_end:
                                col = pieces[pi_][1]
                                lo = pieces[pi_][2]
                                hi = pi_
                                while hi < len(pieces) and pieces[hi][1] == col:
                                    hi += 1
                                hi_r = pieces[hi - 1][3]
                                s = wpool.tile([P, P], mybir.dt.bfloat16,
                                               tag="S")
                                nc.vector.tensor_scalar(
                                    out=s[lo:hi_r, :], in0=iota[lo:hi_r, :],
                                    scalar1=tok_dloc_t[lo:hi_r, col:col + 1],
                                    scalar2=tok_val_t[lo:hi_r, col:col + 1],
                                    op0=mybir.AluOpType.is_equal,
                                    op1=mybir.AluOpType.mult)
                                ci = col - (seg_col0 + rc0)
                                for dwi, _, r0, r1, first, last in pieces[pi_:hi]:
                                    j = dwi - sgdws[0]
                                    if dwi not in ptiles:
                                        ptiles[dwi] = psum_pool.tile(
                                            [P, DIM], f32, tag=f"ps{j}",
                                            name=f"pt{j}")
                                    nc.tensor.matmul(
                                        ptiles[dwi][:], s[r0:r1, :],
                                        g[r0:r1, ci, 0:DIM],
                                        start=first, stop=last)
                                pi_ = hi
                    for dwi in sgdws:
                        ysl = ysum[:, (dwi - DW0[dwc]) * DIM:
                                   (dwi - DW0[dwc] + 1) * DIM]
                        pt = ptiles.get(dwi)
                        if phi == 0:
                            if pt is None:
                                nc.gpsimd.memset(ysl, 0.0)
                            else:
                                nc.scalar.copy(out=ysl, in_=pt[:])
                        else:
                            if pt is not None:
                                nc.vector.tensor_tensor(
                                    out=ysl, in0=ysl, in1=pt[:],
                                    op=mybir.AluOpType.add)
                            if phi == 2:
                                nc.vector.tensor_tensor(
                                    out=accum[:, dwi * DIM:(dwi + 1) * DIM],
                                    in0=accum[:, dwi * DIM:(dwi + 1) * DIM],
                                    in1=ysl, op=mybir.AluOpType.add)
                    if phi == 2:
                        r0 = REG_BASE[dwc] + (sgdws[0] - DW0[dwc]) * P
                        r1_ = r0 + len(sgdws) * P
                        c0 = (sgdws[0] - DW0[dwc]) * DIM
                        c1 = (sgdws[-1] - DW0[dwc] + 1) * DIM
                        nc.gpsimd.dma_start(
                            out=shard_bounce[r0:r1_, 0:DIM].rearrange(
                                "(dw p) j -> p dw j", p=P),
                            in_=ysum[:, c0:c1].rearrange(
                                "p (dw j) -> p dw j", j=DIM))
                    if bi == last_blk_of_dwc[dwc] and layer < num_layers:
                        sl = slice(REG_BASE[dwc], REG_BASE[dwc] + REG_ROWS[dwc])
                        nc.gpsimd.collective_compute(
                            "AllGather", mybir.AluOpType.bypass,
                            replica_groups=rg,
                            ins=[shard_bounce[sl, :].opt()],
                            outs=[tables[layer][dwc][:].opt()])

            # ---- BPR loss1: compact local pooled rows per region, two
            # AllGathers (region-0's hides under the last layer's dwc1
            # compute), then gather triple rows from the compact table ----
            comp_idx_t = pp.tile([P, (S3A + S3B) // 16], mybir.dt.int16,
                                 tag="compidx")
            nc.sync.dma_start(out=comp_idx_t[:], in_=comp_idx[:])
            for dwc, cs, co in ((0, S3A, 0), (1, S3B, S3A)):
                a0 = DW0[dwc] * DIM
                a1 = (DW0[dwc] + REG_DWS[dwc]) * DIM
                nc.vector.tensor_scalar_mul(accum[:, a0:a1], accum[:, a0:a1],
                                            1.0 / (num_layers + 1))
                sl = slice(REG_BASE[dwc], REG_BASE[dwc] + REG_ROWS[dwc])
                nc.sync.dma_start(
                    out=pooled_bounce[sl, :].rearrange("(dw p) j -> p dw j",
                                                       p=P),
                    in_=accum[:, a0:a1].rearrange("p (dw j) -> p dw j", j=DIM))
                ct = bpool.tile([P, max(S3A, S3B) // P, DIM], f32, tag="compt",
                                name="ct")
                o = 0
                while o < cs:
                    n = min(2048, cs - o)
                    nc.gpsimd.dma_gather(
                        ct[:, o // P:(o + n) // P, :], pooled_bounce[:],
                        comp_idx_t[:, (co + o) // 16:(co + o + n) // 16],
                        n, n, DIM, single_packet=False)
                    o += n
                if dwc == 0:
                    nc.sync.dma_start(
                        out=comp_bounce[0:S3A, :].rearrange(
                            "(b p) j -> p b j", p=P),
                        in_=ct[:, :S3A // P, :])
                    nc.gpsimd.collective_compute(
                        "AllGather", mybir.AluOpType.bypass, replica_groups=rg,
                        ins=[comp_bounce[0:S3A, :].opt()],
                        outs=[comp_table[0:NCORES * S3A, :].opt()])
                else:
                    nc.sync.dma_start(
                        out=comp_bounce[S3A:S3A + S3B, :].rearrange(
                            "(b p) j -> p b j", p=P),
                        in_=ct[:, :S3B // P, :])
                    nc.sync.dma_start(
                        out=comp_bounce[S3A + S3B:S3A + S3B + 1, :],
                        in_=regrow[:])
                    nc.gpsimd.collective_compute(
                        "AllGather", mybir.AluOpType.bypass, replica_groups=rg,
                        ins=[comp_bounce[S3A:S3A + S3B + 1, :].opt()],
                        outs=[comp_table[NCORES * S3A:, :].opt()])
            if debug_outputs:
                nc.sync.dma_start(out=dbg["pooled_shard"][:], in_=pooled_bounce[:])

            bidx = {}
            for name, srct in (("u", bpr_u), ("p", bpr_p), ("n", bpr_n)):
                t = pp.tile([P, BATCH // 16], mybir.dt.int16, tag=f"bidx{name}")
                nc.sync.dma_start(out=t[:], in_=srct[:])
                bidx[name] = t

            HB = BATCH // 2          # triples per half-pass
            HNB = HB // P

            def bpr_gather(idx_tile, h, tag):
                out_t = bpool.tile([P, HNB, DIM], f32, tag=tag, name="bg")
                o = 0
                while o < HB:
                    n = min(2048, HB - o)
                    oo = h * HB + o
                    nc.gpsimd.dma_gather(
                        out_t[:, o // P:(o + n) // P, :], comp_table[:],
                        idx_tile[:, oo // 16:(oo + n) // 16],
                        n, n, DIM, single_packet=False)
                    o += n
                return out_t

            ps = pp.tile([P, NB], f32, tag="psc")
            ns = pp.tile([P, NB], f32, tag="nsc")
            for h in (0, 1):
                U = bpr_gather(bidx["u"], h, "bgU")
                Pp = bpr_gather(bidx["p"], h, "bgV")
                tmp = bpool.tile([P, HNB, DIM], f32, tag="tmp")
                nc.vector.tensor_tensor(out=tmp[:], in0=U[:], in1=Pp[:],
                                        op=mybir.AluOpType.mult)
                nc.vector.tensor_reduce(out=ps[:, h * HNB:(h + 1) * HNB],
                                        in_=tmp[:],
                                        axis=mybir.AxisListType.X,
                                        op=mybir.AluOpType.add)
                Nn = bpr_gather(bidx["n"], h, "bgV")
                nc.vector.tensor_tensor(out=tmp[:], in0=U[:], in1=Nn[:],
                                        op=mybir.AluOpType.mult)
                nc.vector.tensor_reduce(out=ns[:, h * HNB:(h + 1) * HNB],
                                        in_=tmp[:],
                                        axis=mybir.AxisListType.X,
                                        op=mybir.AluOpType.add)
            d = pp.tile([P, NB], f32, tag="dsc")
            nc.vector.tensor_tensor(out=d[:], in0=ns[:], in1=ps[:],
                                    op=mybir.AluOpType.subtract)
            # softplus(d) = ln(1 + exp(d)) — Softplus has no ACT table here
            sp = pp.tile([P, NB], f32, tag="spc")
            nc.scalar.activation(sp[:], d[:], mybir.ActivationFunctionType.Exp)
            nc.vector.tensor_scalar_add(sp[:], sp[:], 1.0)
            nc.scalar.activation(sp[:], sp[:], mybir.ActivationFunctionType.Ln)
            s1 = pp.tile([P, 1], f32)
            nc.vector.tensor_reduce(out=s1[:], in_=sp[:],
                                    axis=mybir.AxisListType.X,
                                    op=mybir.AluOpType.add)
            loss_ps = psum_pool.tile([1, 1], f32, tag="ps1")
            nc.tensor.matmul(loss_ps[:], ones[:], s1[:], start=True, stop=True)

            # reg finalize: the 8 per-core partials sit at comp-table row
            # c*(S3+1)+S3 col 0; gather them (16x replicated to 128 idxs),
            # partition-sum via ones-matmul, scale by /16 for the replication.
            reg_slots_t = pp.tile([P, P // 16], mybir.dt.int16, tag="regslots")
            nc.sync.dma_start(out=reg_slots_t[:], in_=reg_slots[:])
            gr = bpool.tile([P, 1, DIM], f32, tag="greg")
            nc.gpsimd.dma_gather(gr[:], comp_table[:], reg_slots_t[:],
                                 P, P, DIM, single_packet=False)
            reg_fin = psum_pool.tile([1, 1], f32, tag="ps2")
            nc.tensor.matmul(reg_fin[:], ones[:], gr[:, 0, 0:1],
                             start=True, stop=True)

            tot = pp.tile([1, 2], f32)
            nc.vector.tensor_scalar_mul(tot[:, 0:1], loss_ps[:], 1.0 / BATCH)
            nc.vector.tensor_scalar_mul(tot[:, 1:2], reg_fin[:],
                                        0.5 / BATCH / 16.0)
            nc.sync.dma_start(out=out_loss[:], in_=tot[:])

    nc.compile()
    _spread_swdge_queues(nc)
    return nc


def _spread_swdge_queues(nc, nq=4):
    """Post-schedule: route each SWDGE op to queue (assigned DMASW lane % nq).

    Tile assigns DMASW completion-sem lanes round-robin over SWDGE ops in
    scheduled order; pairing queue = lane % nq keeps each sem lane locked to
    one queue (required for in-order completion semantics) while spreading
    work over all 4 HW SWDGE queues (~3x gather throughput).
    """
    import re
    pat = re.compile(r"DMASW(\d+)_")
    for bb in nc.main_func.blocks:
        for ins in bb.instructions:
            tn = type(ins).__name__
            if tn not in ("InstDMAGatherAnt", "InstDMACopy"):
                continue
            if tn == "InstDMACopy" and getattr(ins, "queue", None) is not None \
                    and not str(ins.queue).startswith("qPoolDynamic"):
                continue
            if tn == "InstDMACopy" and getattr(ins, "queue", None) is None:
                continue
            si = ins.sync_info
            if not si or not si.on_update:
                continue
            m = pat.match(si.on_update[0].ant_name or "")
            if not m:
                continue
            q = int(m.group(1)) % nq
            if tn == "InstDMAGatherAnt":
                ins.queue_num = q
            else:
                ins.queue = f"qPoolDynamic{q if q else ''}"


# ---------------- public entry point ----------------
def build_for_sim(user_weight, item_weight, edge_vals, edge_row, edge_col,
                  user_index, pos_index, neg_index, num_layers, _debug=False):
    """Build the compiled module + per-core input maps without executing."""
    return _prepare(user_weight, item_weight, edge_vals, edge_row, edge_col,
                    user_index, pos_index, neg_index, num_layers, _debug)


def _prepare(user_weight, item_weight, edge_vals, edge_row, edge_col,
             user_index, pos_index, neg_index, num_layers, _debug=False):
    user_weight = np.asarray(user_weight, dtype=np.float32)
    item_weight = np.asarray(item_weight, dtype=np.float32)
    edge_vals = np.asarray(edge_vals, dtype=np.float32)
    edge_row = np.asarray(edge_row, dtype=np.int64)
    edge_col = np.asarray(edge_col, dtype=np.int64)
    user_index = np.asarray(user_index, dtype=np.int64)
    pos_index = np.asarray(pos_index, dtype=np.int64)
    neg_index = np.asarray(neg_index, dtype=np.int64)
    L = int(num_layers)

    pre = _preprocess(edge_row, edge_col, edge_vals)
    core_n, loc_n = pre["core"], pre["loc"]

    x0_nodes = np.concatenate([user_weight, item_weight], axis=0)
    x0_shards = np.zeros((NCORES, SHARD, DIM), np.float32)
    x0_shards[core_n, loc_n] = x0_nodes

    iota = np.tile(np.arange(P, dtype=np.float32),
                   (P, 1)).astype(ml_dtypes.bfloat16)

    # BPR pooled-side compaction: per-core unique local rows referenced
    node_u = user_index
    node_p = NUM_USERS + pos_index
    node_n = NUM_USERS + neg_index
    trip_nodes = {"u": node_u, "p": node_p, "n": node_n}
    allk = np.unique(np.concatenate(
        [core_n[v] * SHARD + loc_n[v] for v in trip_nodes.values()]))
    core_of = allk // SHARD
    within = allk % SHARD
    uniq0 = [within[(core_of == c) & (within < REG_BASE[1])]
             for c in range(NCORES)]
    uniq1 = [within[(core_of == c) & (within >= REG_BASE[1])]
             for c in range(NCORES)]
    S3A = max(128, -(-max(len(x) for x in uniq0) // 128) * 128)
    S3B = max(128, -(-max(len(x) for x in uniq1) // 128) * 128)
    assert NCORES * S3A + NCORES * (S3B + 1) < 32768, (S3A, S3B)
    comp_idx_arr = []
    slot_map = np.zeros(NCORES * SHARD, dtype=np.int64)
    for c in range(NCORES):
        pad = np.zeros(S3A + S3B, np.int64)
        pad[:len(uniq0[c])] = uniq0[c]
        pad[S3A:S3A + len(uniq1[c])] = uniq1[c]
        comp_idx_arr.append(_idx16_layout(pad.astype(np.int16)))
        slot_map[c * SHARD + uniq0[c]] = c * S3A + np.arange(len(uniq0[c]))
        slot_map[c * SHARD + uniq1[c]] = (NCORES * S3A + c * (S3B + 1)
                                          + np.arange(len(uniq1[c])))
    b_tok = {k: _idx16_layout(
        slot_map[core_n[v] * SHARD + loc_n[v]].astype(np.int16))
        for k, v in trip_nodes.items()}

    # L2-reg multiplicities: count of each local row among the 3*BATCH samples
    all_nodes = np.concatenate([node_u, node_p, node_n])
    M = np.zeros((NCORES, SHARD), np.float32)
    np.add.at(M, (core_n[all_nodes], loc_n[all_nodes]), 1.0)
    reg_mult_arr = [M[c].reshape(NDW, P).T.copy() for c in range(NCORES)]
    reg_slot_list = np.array(
        [NCORES * S3A + c * (S3B + 1) + S3B for c in range(NCORES)], np.int64)
    reg_slots_arr = _idx16_layout(
        np.tile(reg_slot_list, P // NCORES).astype(np.int16))

    nc = _build_kernel(L, pre["sched"], pre["toktot"], S3A, S3B,
                       debug_outputs=_debug)

    in_maps = []
    for c in range(NCORES):
        in_maps.append({
            "x0_shard": x0_shards[c],
            "tok_idx": _idx16_layout(pre["sloc"][c]),
            "tok_val": _pm_layout(pre["val"][c]),
            "tok_dloc": _pm_layout(pre["dloc"][c]),
            "iota_in": iota,
            "bpr_u": b_tok["u"], "bpr_p": b_tok["p"], "bpr_n": b_tok["n"],
            "comp_idx": comp_idx_arr[c],
            "reg_mult": reg_mult_arr[c],
            "reg_slots": reg_slots_arr,
        })
    _prepare.last_maps = (core_n, loc_n)
    return nc, in_maps


def kernel(user_weight, item_weight, edge_vals, edge_row, edge_col,
           user_index, pos_index, neg_index, num_layers, _debug=False):
    nc, in_maps = _prepare(user_weight, item_weight, edge_vals, edge_row,
                           edge_col, user_index, pos_index, neg_index,
                           num_layers, _debug)
    from concourse.bass_utils import run_bass_kernel_spmd
    kernel._cache = (nc, in_maps)
    res = run_bass_kernel_spmd(nc, in_maps, core_ids=list(range(NCORES)))
    out = res.results[0]["out_loss"]
    loss1 = np.float32(out[0, 0])
    reg = np.float32(out[0, 1])
    if _debug:
        pooled = np.stack(
            [res.results[c]["pooled_shard_out"] for c in range(NCORES)], axis=0)
        kernel._debug_pooled = (pooled, _prepare.last_maps)
    return loss1, reg


# revision 44
# speedup vs baseline: 6.5098x; 6.5098x over previous
"""LightGCN message-passing + BPR loss on 8 Trainium2 NeuronCores.

Dest-sharded SpMM with pipelined AllGathers. Nodes are permuted into 8 core
shards (dealt round-robin by degree), each shard split into TWO row regions
(72 + 75 dest-windows of 128 rows). The inter-layer exchange is TWO
AllGather collectives (one per region) so each layer's compute overlaps the
other region's collective: every layer runs as region-phases — phase p of
any dest-window only gathers source embeddings from source-region p, so
phase p of layer l+1 depends only on AG chunk p of layer l.

Per 128-token chunk a selection matrix S[token, dest_local] =
val(token) * (dlocal(token) == dest_local) is built with one DVE
tensor_scalar op and matmul-accumulated into the dest window's PSUM tile
(one PSUM accumulation group per phase; phases summed via an SBUF ysum
tile). The BPR head computes the L2-reg term fully locally (each sampled
row is owned by exactly one core) followed by a tiny scalar AllReduce that
runs under the layer compute; only the pooled-embedding compact+AllGather
remains on the critical tail.

Returns (loss1, reg_loss) like the reference.
"""
import sys

sys.path.insert(0, "/opt/trn_rl_repo")

import numpy as np
import ml_dtypes

import concourse.bass as bass
import concourse.bacc as bacc
import concourse.tile as tile
from concourse import mybir, library_config

# ---------------- problem constants (hardcoded per spec) ----------------
NUM_USERS = 100000
NUM_ITEMS = 50000
DIM = 64
BATCH = 8192
NCORES = 8

P = 128                      # partitions / rows per dest window
NDW = 147                    # dest windows per core
SHARD = NDW * P              # 18816 rows per core
NREG = 3
REG_DWS = (49, 49, 49)       # dest windows per region
DW0 = (0, 49, 98)            # first dw of each region
REG_ROWS = (6272, 6272, 6272)                          # rows per core
REG_BASE = (0, 6272, 12544)                            # local row base
REG_GROWS = (50176, 50176, 50176)
REG_GBASE = (0, 50176, 100352)
NTOT_G = 150528
# source windows: 2 per region, int16-addressable
WIN_SIZE = (25088,) * 6
WIN_REG = (0, 0, 1, 1, 2, 2)
WIN_LBASE = (0, 25088, 0, 25088, 0, 25088)             # base within region
NSW = 6
# phases per dw-chunk: phase p gathers from source-region p, so phase p of
# layer l+1 depends only on AG chunk p of layer l
PHASES = ((0, 1), (2, 3), (4, 5))
# block emission order: phase-0 blocks first (they consume AG chunk 0 of the
# previous layer), then finish region 0 early so its AG chunk is in flight
# well before the next layer needs it
BLK_ORDER = ((0, 0), (1, 0), (2, 0), (0, 1), (1, 1),
             (0, 2), (2, 1), (1, 2), (2, 2))
SG = 7                       # dest windows per supergroup (PSUM banks)
MAXCH = 16                   # chunks (128 tokens each) per dma_gather

SG_BLOCKS = tuple(
    [list(range(s, s + SG)) for s in range(DW0[r], DW0[r] + REG_DWS[r], SG)]
    for r in range(NREG)
)


def _win_of_core_dw():
    """Source window as a function of (core, dw) — boundaries are dw-aligned."""
    w = np.zeros((NCORES, NDW), np.int64)
    for c in range(NCORES):
        for k in range(NDW):
            r = k // REG_DWS[0]
            g0 = c * REG_ROWS[r] + (k - DW0[r]) * P
            w[c, k] = 2 * r + g0 // 25088
    return w


# ---------------- host-side graph preprocessing ----------------
def _preprocess(edge_row, edge_col, edge_vals):
    n_nodes = NUM_USERS + NUM_ITEMS
    deg = np.bincount(edge_row, minlength=n_nodes)

    # Deal degree-sorted nodes round-robin over the 1176 (core, dw) slots in
    # snake order; partition lane = deal round.
    order = np.argsort(-deg, kind="stable")
    nslots = NCORES * NDW
    idx = np.arange(n_nodes)
    rounds = idx // nslots
    within = idx % nslots
    snake = np.where(rounds % 2 == 0, within, nslots - 1 - within)
    slot_ids = np.empty(n_nodes, dtype=np.int64)
    lane = np.empty(n_nodes, dtype=np.int64)
    slot_ids[order] = snake
    lane[order] = rounds
    core = slot_ids // NDW
    dw = slot_ids % NDW

    # Window-balancing rebalance: the shared SPMD schedule pads every
    # (dw, window) group to the max count over cores, so imbalance of the
    # per-core window profiles is pure padding. Within each dw, reassign its
    # 1024 nodes across cores (128 each) by quadratic greedy on the nodes'
    # in-edge source-window profiles. Two passes absorb the feedback of
    # sources moving between windows.
    wmap = _win_of_core_dw()
    for _ in range(2):
        w_edge = wmap[core[edge_col], dw[edge_col]]
        prof = np.zeros((n_nodes, NSW), np.float64)
        np.add.at(prof, (edge_row, w_edge), 1.0)
        new_core = np.empty(n_nodes, dtype=np.int64)
        new_lane = np.empty(n_nodes, dtype=np.int64)
        for k in range(NDW):
            nodes_k = np.flatnonzero(dw == k)
            pk = prof[nodes_k]
            ordk = np.argsort(-pk.sum(axis=1), kind="stable")
            nodes_k = nodes_k[ordk]
            pk = pk[ordk]
            cnt8 = np.zeros((NCORES, NSW), np.float64)
            fill = np.zeros(NCORES, np.int64)
            for i in range(len(nodes_k)):
                tot = cnt8 + pk[i]
                score = (tot * tot).sum(axis=1)
                score[fill >= P] = np.inf
                c = int(np.argmin(score))
                cnt8[c] += pk[i]
                new_core[nodes_k[i]] = c
                new_lane[nodes_k[i]] = fill[c]
                fill[c] += 1
        core = new_core
        lane = new_lane
    regn = dw // REG_DWS[0]
    loc = np.choose(regn, REG_BASE) + (dw - np.choose(regn, DW0)) * P + lane
    gpos = np.choose(regn, REG_GBASE) + core * 6272 \
        + (dw - np.choose(regn, DW0)) * P + lane

    # token fields
    d_core = core[edge_row]
    d_dw = dw[edge_row]
    d_part = lane[edge_row]
    s_g = gpos[edge_col]
    s_reg = s_g // 50176
    s_in = s_g - s_reg * 50176
    t_sw = 2 * s_reg + s_in // 25088
    t_sloc = s_in % 25088
    t_val = np.asarray(edge_vals, dtype=np.float32)

    # group rank order = block order, block = (dwc, phase, sg)
    NGR = NDW * NSW
    g_local = d_dw * NSW + t_sw
    ranked_groups = []
    for dwc, phi in BLK_ORDER:
        for sgdws in SG_BLOCKS[dwc]:
            for w in PHASES[phi]:
                for dwi in sgdws:
                    ranked_groups.append(dwi * NSW + w)
    ranked_groups = np.asarray(ranked_groups)
    rank_of_g = np.empty(NGR, dtype=np.int64)
    rank_of_g[ranked_groups] = np.arange(NGR)

    # counts per (core, group); shared schedule = max over cores, padded to
    # 16 tokens per group (idx-layout granularity); each (block, window)
    # segment padded to 128 so gather runs stay column-aligned
    cnt = np.zeros((NCORES, NGR), dtype=np.int64)
    np.add.at(cnt, (d_core, g_local), 1)
    n16_g = -(-cnt.max(axis=0) // 32) * 32         # [NGR] natural order
    grp_off = np.zeros(NGR, dtype=np.int64)
    seg_layout = {}                                # (blk_idx, w) -> (off, ncols)
    cur = 0
    bi = 0
    blk_bounds = []                                # (off, ntok) per block
    for dwc, phi in BLK_ORDER:
        for sgdws in SG_BLOCKS[dwc]:
            blk_off = cur
            for w in PHASES[phi]:
                seg_off = cur
                for dwi in sgdws:
                    g = dwi * NSW + w
                    grp_off[g] = cur
                    cur += int(n16_g[g])
                    if cur % P == 96:
                        # PE matmul base partition must be 0/32/64 — never
                        # start a group at row 96 of a column
                        cur += 32
                cur = -(-cur // P) * P
                seg_layout[(bi, w)] = (seg_off, (cur - seg_off) // P)
            blk_bounds.append((blk_off, cur - blk_off))
            bi += 1
    toktot = int(cur)

    # within-group index per token (stable order)
    K = d_core * NGR + rank_of_g[g_local]
    perm = np.argsort(K, kind="stable")
    Ks = K[perm]
    starts = np.r_[0, np.flatnonzero(np.diff(Ks)) + 1]
    grp_start = starts[np.searchsorted(Ks[starts], Ks)]
    within_grp = np.arange(len(Ks)) - grp_start
    tgt = grp_off[g_local[perm]] + within_grp
    c_perm = d_core[perm]

    sloc_arr = np.zeros((NCORES, toktot), dtype=np.int16)
    val_arr = np.zeros((NCORES, toktot), dtype=np.float32)
    dloc_arr = np.zeros((NCORES, toktot), dtype=np.float32)
    sloc_arr[c_perm, tgt] = t_sloc[perm].astype(np.int16)
    val_arr[c_perm, tgt] = t_val[perm]
    dloc_arr[c_perm, tgt] = d_part[perm].astype(np.float32)

    # schedule: blocks in processing order; each block = one (dwc, phi, sg).
    # Pieces are (dw, col, r0, r1) row-slices of 128-token columns — a
    # column can host (parts of) several 16-granular groups.
    sched = []
    bi = 0
    for dwc, phi in BLK_ORDER:
        for sgdws in SG_BLOCKS[dwc]:
            blk_off, blk_ntok = blk_bounds[bi]
            npieces = {d: 0 for d in sgdws}
            segs_raw = []
            for w in PHASES[phi]:
                seg_off, seg_ncols = seg_layout[(bi, w)]
                pieces = []
                for dwi in sgdws:
                    g = dwi * NSW + w
                    o = int(grp_off[g])
                    rem = int(n16_g[g])
                    while rem > 0:
                        r0 = o % P
                        take = min(P - r0, rem)
                        # split to PE-quadrant-legal (base, size) sub-pieces:
                        # base 0 any size, base 64 up to 64, base 32/96 up
                        # to 32
                        a, b = r0, r0 + take
                        while a < b:
                            if a == 0:
                                c = b
                            elif a == 32:
                                c = min(b, 64)
                            else:
                                assert a == 64, a
                                c = b
                            pieces.append([dwi, o // P, a, c])
                            npieces[dwi] += 1
                            a = c
                        o += take
                        rem -= take
                pieces.sort(key=lambda x: (x[1], x[2]))
                segs_raw.append((w, seg_off, seg_ncols, pieces))
            seen = {d: 0 for d in sgdws}
            segs = []
            for w, seg_off, seg_ncols, pieces in segs_raw:
                out = []
                for dwi, col, r0, r1 in pieces:
                    seen[dwi] += 1
                    out.append((dwi, col, r0, r1, seen[dwi] == 1,
                                seen[dwi] == npieces[dwi]))
                segs.append((w, seg_off, seg_ncols, out))
            sched.append(dict(dwc=dwc, phi=phi, dws=sgdws, off=blk_off,
                              ntok=blk_ntok, segs=segs))
            bi += 1

    return dict(core=core, loc=loc, sloc=sloc_arr, val=val_arr, dloc=dloc_arr,
                toktot=toktot, sched=sched)


def _idx16_layout(sloc_row):
    """int16 token array -> dma_gather idxs layout [128, n/16] (8x replicated)."""
    n = sloc_row.shape[0]
    a = np.zeros((16, n // 16), np.int16)
    a[np.arange(n) % 16, np.arange(n) // 16] = sloc_row
    return np.tile(a, (8, 1))


def _pm_layout(arr_row):
    """token array -> [128, n/128] (token t at [t%128, t//128])."""
    n = arr_row.shape[0]
    a = np.zeros((P, n // P), arr_row.dtype)
    a[np.arange(n) % P, np.arange(n) // P] = arr_row
    return a


# ---------------- device kernel ----------------
def _build_kernel(num_layers, sched, toktot, S3A, S3B, debug_outputs=False):
    nc = bacc.Bacc(None, target_bir_lowering=False, num_swdge_queues=4)
    f32 = mybir.dt.float32
    NB = BATCH // P
    max_blktok = max(b["ntok"] for b in sched)
    rg = [list(range(NCORES))]

    x0_shard = nc.dram_tensor("x0_shard", [SHARD, DIM], f32, kind="ExternalInput")
    tok_idx = nc.dram_tensor("tok_idx", [P, toktot // 16], mybir.dt.int16,
                             kind="ExternalInput")
    tok_val = nc.dram_tensor("tok_val", [P, toktot // P], f32, kind="ExternalInput")
    tok_dloc = nc.dram_tensor("tok_dloc", [P, toktot // P], f32, kind="ExternalInput")
    iota_in = nc.dram_tensor("iota_in", [P, P], mybir.dt.bfloat16,
                             kind="ExternalInput")
    bpr_u = nc.dram_tensor("bpr_u", [P, BATCH // 16], mybir.dt.int16, kind="ExternalInput")
    bpr_p = nc.dram_tensor("bpr_p", [P, BATCH // 16], mybir.dt.int16, kind="ExternalInput")
    bpr_n = nc.dram_tensor("bpr_n", [P, BATCH // 16], mybir.dt.int16, kind="ExternalInput")
    comp_idx = nc.dram_tensor("comp_idx", [P, (S3A + S3B) // 16],
                              mybir.dt.int16, kind="ExternalInput")
    reg_mult = nc.dram_tensor("reg_mult", [P, NDW], mybir.dt.float32,
                              kind="ExternalInput")
    reg_slots = nc.dram_tensor("reg_slots", [P, P // 16], mybir.dt.int16,
                               kind="ExternalInput")
    out_loss = nc.dram_tensor("out_loss", [1, 2], f32, kind="ExternalOutput")
    dbg = {}
    if debug_outputs:
        dbg["pooled_shard"] = nc.dram_tensor("pooled_shard_out", [SHARD, DIM],
                                             f32, kind="ExternalOutput")

    with tile.TileContext(nc) as tc:
        with (
            tc.tile_pool(name="persist", bufs=1) as pp,
            tc.tile_pool(name="idxs", bufs=2) as ipool,
            tc.tile_pool(name="gath", bufs=6) as gpool,
            tc.tile_pool(name="work", bufs=3) as wpool,
            tc.tile_pool(name="ys", bufs=1) as ypool,
            tc.tile_pool(name="bpr", bufs=1) as bpool,
            tc.tile_pool(name="psum", bufs=1, space="PSUM") as psum_pool,
            tc.tile_pool(name="dram", bufs=1, space="DRAM") as dram,
        ):
            with tc.tile_critical():
                nc.gpsimd.load_library(library_config.mlp)

            # per-layer tables, one DRAM tensor per region so Tile tracks
            # the two AG chunks as independent dependencies
            bf16 = mybir.dt.bfloat16
            tables = []
            for l in range(num_layers + 1):
                ts = []
                for r in range(NREG):
                    t_ = dram.tile([REG_GROWS[r], 2 * DIM], bf16,
                                   tag=f"table{l}r{r}", name=f"t{r}")
                    ts.append(t_)
                tables.append(tuple(ts))
            shard_bounce = dram.tile([SHARD, 2 * DIM], bf16)
            pooled_bounce = dram.tile([SHARD, DIM], f32)
            comp_bounce = dram.tile([S3A + S3B + 1, DIM], f32)
            comp_table = dram.tile(
                [NCORES * S3A + NCORES * (S3B + 1), DIM], f32)

            iota = pp.tile([P, P], mybir.dt.bfloat16)
            nc.sync.dma_start(out=iota[:], in_=iota_in[:])

            accum = pp.tile([P, NDW * DIM], f32)
            nc.sync.dma_start(
                out=accum[:].rearrange("p (dw j) -> p dw j", j=DIM),
                in_=x0_shard[:].rearrange("(dw p) j -> p dw j", p=P))

            # x0 -> bounce -> per-region AllGather into table0
            for dwc in range(NREG):
                sl = slice(REG_BASE[dwc], REG_BASE[dwc] + REG_ROWS[dwc])
                nc.gpsimd.dma_start(out=shard_bounce[sl, 0:DIM],
                                    in_=x0_shard[sl, :])
                nc.gpsimd.collective_compute(
                    "AllGather", mybir.AluOpType.bypass, replica_groups=rg,
                    ins=[shard_bounce[sl, :].opt()],
                    outs=[tables[0][dwc][:].opt()])

            tok_val_t = pp.tile([P, toktot // P], f32)
            nc.sync.dma_start(out=tok_val_t[:], in_=tok_val[:])
            tok_dloc_t = pp.tile([P, toktot // P], f32)
            nc.sync.dma_start(out=tok_dloc_t[:], in_=tok_dloc[:])

            ones = pp.tile([P, 1], f32)
            nc.gpsimd.memset(ones[:], 1.0)

            # ---- L2-reg partial: multiplicity-weighted local square sum ----
            # reg = sum over sampled rows of |x0|^2 = sum_r mult(r) * |x0_r|^2.
            # Every sampled row is owned by exactly one core; accum == x0 at
            # this point. Partial travels as an extra row of the comp table's
            # AllGather (no extra collective, no gathers).
            reg_mult_t = pp.tile([P, NDW], f32)
            nc.sync.dma_start(out=reg_mult_t[:], in_=reg_mult[:])
            sq = ypool.tile([P, REG_DWS[0] * DIM], f32, tag="ysum2", name="sq")
            rr = pp.tile([P, 1], f32)
            base = 0
            for k, nd in enumerate((49, 49, 49)):
                c0, c1 = base * DIM, (base + nd) * DIM
                nc.vector.tensor_tensor(out=sq[:, :nd * DIM],
                                        in0=accum[:, c0:c1],
                                        in1=accum[:, c0:c1],
                                        op=mybir.AluOpType.mult)
                r1 = pp.tile([P, 49], f32, tag="regr1", name="r1")
                nc.vector.tensor_reduce(
                    out=r1[:, :nd],
                    in_=sq[:, :nd * DIM].rearrange("p (d j) -> p d j", j=DIM),
                    axis=mybir.AxisListType.X, op=mybir.AluOpType.add)
                nc.vector.tensor_tensor(out=r1[:, :nd], in0=r1[:, :nd],
                                        in1=reg_mult_t[:, base:base + nd],
                                        op=mybir.AluOpType.mult)
                r2 = pp.tile([P, 1], f32, tag="regr2", name="r2")
                nc.vector.tensor_reduce(out=r2[:], in_=r1[:, :nd],
                                        axis=mybir.AxisListType.X,
                                        op=mybir.AluOpType.add)
                if k == 0:
                    nc.vector.tensor_copy(out=rr[:], in_=r2[:])
                else:
                    nc.vector.tensor_tensor(out=rr[:], in0=rr[:], in1=r2[:],
                                            op=mybir.AluOpType.add)
                base += nd
            reg_ps = psum_pool.tile([1, 1], f32, tag="ps0")
            nc.tensor.matmul(reg_ps[:], ones[:], rr[:], start=True, stop=True)
            regrow = pp.tile([1, DIM], f32)
            nc.gpsimd.memset(regrow[:], 0.0)
            nc.scalar.copy(out=regrow[:, 0:1], in_=reg_ps[:])

            # ---- layers ----
            for layer in range(1, num_layers + 1):
                src = tables[layer - 1]
                ysums = {}
                last_blk_of_dwc = {dwc: max(i for i, b in enumerate(sched)
                                            if b["dwc"] == dwc)
                                   for dwc in range(NREG)}
                for bi, blk in enumerate(sched):
                    dwc = blk["dwc"]
                    phi = blk["phi"]
                    sgdws = blk["dws"]
                    if dwc not in ysums:
                        ysums[dwc] = ypool.tile(
                            [P, REG_DWS[dwc] * DIM], f32, tag=f"ysum{dwc}",
                            name=f"ysum{dwc}")
                    ysum = ysums[dwc]
                    sg_idx = ipool.tile([P, max_blktok // 16],
                                        mybir.dt.int16, tag="sgidx")
                    nc.sync.dma_start(
                        out=sg_idx[:, : blk["ntok"] // 16],
                        in_=tok_idx[:, blk["off"] // 16:
                                    (blk["off"] + blk["ntok"]) // 16])
                    ptiles = {}
                    for w, seg_off, seg_ncols, pieces in blk["segs"]:
                        src_win = src[WIN_REG[w]][
                            WIN_LBASE[w]:WIN_LBASE[w] + WIN_SIZE[w], :]
                        seg_col0 = seg_off // P
                        pi_ = 0
                        for rc0 in range(0, seg_ncols, MAXCH):
                            ncols = min(MAXCH, seg_ncols - rc0)
                            ntok = ncols * P
                            g = gpool.tile([P, MAXCH, 2 * DIM],
                                           mybir.dt.bfloat16, tag="g")
                            locw = (seg_off + rc0 * P - blk["off"]) // 16
                            nc.gpsimd.dma_gather(
                                g[:, :ncols, :], src_win,
                                sg_idx[:, locw:locw + ntok // 16],
                                ntok, ntok, 2 * DIM, single_packet=False)
                            run_end = seg_col0 + rc0 + ncols
                            while pi_ < len(pieces) and pieces[pi_][1] < run_end:
                                col = pieces[pi_][1]
                                lo = pieces[pi_][2]
                                hi = pi_
                                while hi < len(pieces) and pieces[hi][1] == col:
                                    hi += 1
                                hi_r = pieces[hi - 1][3]
                                s = wpool.tile([P, P], mybir.dt.bfloat16,
                                               tag="S")
                                nc.vector.tensor_scalar(
                                    out=s[lo:hi_r, :], in0=iota[lo:hi_r, :],
                                    scalar1=tok_dloc_t[lo:hi_r, col:col + 1],
                                    scalar2=tok_val_t[lo:hi_r, col:col + 1],
                                    op0=mybir.AluOpType.is_equal,
                                    op1=mybir.AluOpType.mult)
                                ci = col - (seg_col0 + rc0)
                                for dwi, _, r0, r1, first, last in pieces[pi_:hi]:
                                    j = dwi - sgdws[0]
                                    if dwi not in ptiles:
                                        ptiles[dwi] = psum_pool.tile(
                                            [P, DIM], f32, tag=f"ps{j}",
                                            name=f"pt{j}")
                                    nc.tensor.matmul(
                                        ptiles[dwi][:], s[r0:r1, :],
                                        g[r0:r1, ci, 0:DIM],
                                        start=first, stop=last)
                                pi_ = hi
                    for dwi in sgdws:
                        ysl = ysum[:, (dwi - DW0[dwc]) * DIM:
                                   (dwi - DW0[dwc] + 1) * DIM]
                        pt = ptiles.get(dwi)
                        if phi == 0:
                            if pt is None:
                                nc.gpsimd.memset(ysl, 0.0)
                            else:
                                nc.scalar.copy(out=ysl, in_=pt[:])
                        else:
                            if pt is not None:
                                nc.vector.tensor_tensor(
                                    out=ysl, in0=ysl, in1=pt[:],
                                    op=mybir.AluOpType.add)
                            if phi == NREG - 1:
                                nc.vector.tensor_tensor(
                                    out=accum[:, dwi * DIM:(dwi + 1) * DIM],
                                    in0=accum[:, dwi * DIM:(dwi + 1) * DIM],
                                    in1=ysl, op=mybir.AluOpType.add)
                    if phi == NREG - 1:
                        r0 = REG_BASE[dwc] + (sgdws[0] - DW0[dwc]) * P
                        r1_ = r0 + len(sgdws) * P
                        c0 = (sgdws[0] - DW0[dwc]) * DIM
                        c1 = (sgdws[-1] - DW0[dwc] + 1) * DIM
                        nc.gpsimd.dma_start(
                            out=shard_bounce[r0:r1_, 0:DIM].rearrange(
                                "(dw p) j -> p dw j", p=P),
                            in_=ysum[:, c0:c1].rearrange(
                                "p (dw j) -> p dw j", j=DIM))
                    if bi == last_blk_of_dwc[dwc] and layer < num_layers:
                        sl = slice(REG_BASE[dwc], REG_BASE[dwc] + REG_ROWS[dwc])
                        nc.gpsimd.collective_compute(
                            "AllGather", mybir.AluOpType.bypass,
                            replica_groups=rg,
                            ins=[shard_bounce[sl, :].opt()],
                            outs=[tables[layer][dwc][:].opt()])

            # ---- BPR loss1: compact local pooled rows per region, two
            # AllGathers (region-0's hides under the last layer's dwc1
            # compute), then gather triple rows from the compact table ----
            comp_idx_t = pp.tile([P, (S3A + S3B) // 16], mybir.dt.int16,
                                 tag="compidx")
            nc.sync.dma_start(out=comp_idx_t[:], in_=comp_idx[:])
            for part, cs, co in ((0, S3A, 0), (1, S3B, S3A)):
                if part == 0:
                    a0, a1 = 0, DW0[2] * DIM
                    sl = slice(0, REG_BASE[2])
                else:
                    a0, a1 = DW0[2] * DIM, NDW * DIM
                    sl = slice(REG_BASE[2], SHARD)
                nc.vector.tensor_scalar_mul(accum[:, a0:a1], accum[:, a0:a1],
                                            1.0 / (num_layers + 1))
                nc.sync.dma_start(
                    out=pooled_bounce[sl, :].rearrange("(dw p) j -> p dw j",
                                                       p=P),
                    in_=accum[:, a0:a1].rearrange("p (dw j) -> p dw j", j=DIM))
                ct = bpool.tile([P, max(S3A, S3B) // P, DIM], f32, tag="compt",
                                name="ct")
                o = 0
                while o < cs:
                    n = min(2048, cs - o)
                    nc.gpsimd.dma_gather(
                        ct[:, o // P:(o + n) // P, :], pooled_bounce[:],
                        comp_idx_t[:, (co + o) // 16:(co + o + n) // 16],
                        n, n, DIM, single_packet=False)
                    o += n
                if part == 0:
                    nc.sync.dma_start(
                        out=comp_bounce[0:S3A, :].rearrange(
                            "(b p) j -> p b j", p=P),
                        in_=ct[:, :S3A // P, :])
                    nc.gpsimd.collective_compute(
                        "AllGather", mybir.AluOpType.bypass, replica_groups=rg,
                        ins=[comp_bounce[0:S3A, :].opt()],
                        outs=[comp_table[0:NCORES * S3A, :].opt()])
                else:
                    nc.sync.dma_start(
                        out=comp_bounce[S3A:S3A + S3B, :].rearrange(
                            "(b p) j -> p b j", p=P),
                        in_=ct[:, :S3B // P, :])
                    nc.sync.dma_start(
                        out=comp_bounce[S3A + S3B:S3A + S3B + 1, :],
                        in_=regrow[:])
                    nc.gpsimd.collective_compute(
                        "AllGather", mybir.AluOpType.bypass, replica_groups=rg,
                        ins=[comp_bounce[S3A:S3A + S3B + 1, :].opt()],
                        outs=[comp_table[NCORES * S3A:, :].opt()])
            if debug_outputs:
                nc.sync.dma_start(out=dbg["pooled_shard"][:], in_=pooled_bounce[:])

            bidx = {}
            for name, srct in (("u", bpr_u), ("p", bpr_p), ("n", bpr_n)):
                t = pp.tile([P, BATCH // 16], mybir.dt.int16, tag=f"bidx{name}")
                nc.sync.dma_start(out=t[:], in_=srct[:])
                bidx[name] = t

            HB = BATCH // 2          # triples per half-pass
            HNB = HB // P

            def bpr_gather(idx_tile, h, tag):
                out_t = bpool.tile([P, HNB, DIM], f32, tag=tag, name="bg")
                o = 0
                while o < HB:
                    n = min(2048, HB - o)
                    oo = h * HB + o
                    nc.gpsimd.dma_gather(
                        out_t[:, o // P:(o + n) // P, :], comp_table[:],
                        idx_tile[:, oo // 16:(oo + n) // 16],
                        n, n, DIM, single_packet=False)
                    o += n
                return out_t

            ps = pp.tile([P, NB], f32, tag="psc")
            ns = pp.tile([P, NB], f32, tag="nsc")
            for h in (0, 1):
                U = bpr_gather(bidx["u"], h, "bgU")
                Pp = bpr_gather(bidx["p"], h, "bgV")
                tmp = bpool.tile([P, HNB, DIM], f32, tag="tmp")
                nc.vector.tensor_tensor(out=tmp[:], in0=U[:], in1=Pp[:],
                                        op=mybir.AluOpType.mult)
                nc.vector.tensor_reduce(out=ps[:, h * HNB:(h + 1) * HNB],
                                        in_=tmp[:],
                                        axis=mybir.AxisListType.X,
                                        op=mybir.AluOpType.add)
                Nn = bpr_gather(bidx["n"], h, "bgV")
                nc.vector.tensor_tensor(out=tmp[:], in0=U[:], in1=Nn[:],
                                        op=mybir.AluOpType.mult)
                nc.vector.tensor_reduce(out=ns[:, h * HNB:(h + 1) * HNB],
                                        in_=tmp[:],
                                        axis=mybir.AxisListType.X,
                                        op=mybir.AluOpType.add)
            d = pp.tile([P, NB], f32, tag="dsc")
            nc.vector.tensor_tensor(out=d[:], in0=ns[:], in1=ps[:],
                                    op=mybir.AluOpType.subtract)
            # softplus(d) = ln(1 + exp(d)) — Softplus has no ACT table here
            sp = pp.tile([P, NB], f32, tag="spc")
            nc.scalar.activation(sp[:], d[:], mybir.ActivationFunctionType.Exp)
            nc.vector.tensor_scalar_add(sp[:], sp[:], 1.0)
            nc.scalar.activation(sp[:], sp[:], mybir.ActivationFunctionType.Ln)
            s1 = pp.tile([P, 1], f32)
            nc.vector.tensor_reduce(out=s1[:], in_=sp[:],
                                    axis=mybir.AxisListType.X,
                                    op=mybir.AluOpType.add)
            loss_ps = psum_pool.tile([1, 1], f32, tag="ps1")
            nc.tensor.matmul(loss_ps[:], ones[:], s1[:], start=True, stop=True)

            # reg finalize: the 8 per-core partials sit at comp-table row
            # c*(S3+1)+S3 col 0; gather them (16x replicated to 128 idxs),
            # partition-sum via ones-matmul, scale by /16 for the replication.
            reg_slots_t = pp.tile([P, P // 16], mybir.dt.int16, tag="regslots")
            nc.sync.dma_start(out=reg_slots_t[:], in_=reg_slots[:])
            gr = bpool.tile([P, 1, DIM], f32, tag="greg")
            nc.gpsimd.dma_gather(gr[:], comp_table[:], reg_slots_t[:],
                                 P, P, DIM, single_packet=False)
            reg_fin = psum_pool.tile([1, 1], f32, tag="ps2")
            nc.tensor.matmul(reg_fin[:], ones[:], gr[:, 0, 0:1],
                             start=True, stop=True)

            tot = pp.tile([1, 2], f32)
            nc.vector.tensor_scalar_mul(tot[:, 0:1], loss_ps[:], 1.0 / BATCH)
            nc.vector.tensor_scalar_mul(tot[:, 1:2], reg_fin[:],
                                        0.5 / BATCH / 16.0)
            nc.sync.dma_start(out=out_loss[:], in_=tot[:])

    nc.compile()
    _spread_swdge_queues(nc)
    return nc


def _spread_swdge_queues(nc, nq=4):
    """Post-schedule: route each SWDGE op to queue (assigned DMASW lane % nq).

    Tile assigns DMASW completion-sem lanes round-robin over SWDGE ops in
    scheduled order; pairing queue = lane % nq keeps each sem lane locked to
    one queue (required for in-order completion semantics) while spreading
    work over all 4 HW SWDGE queues (~3x gather throughput).
    """
    import re
    pat = re.compile(r"DMASW(\d+)_")
    for bb in nc.main_func.blocks:
        for ins in bb.instructions:
            tn = type(ins).__name__
            if tn not in ("InstDMAGatherAnt", "InstDMACopy"):
                continue
            if tn == "InstDMACopy" and getattr(ins, "queue", None) is not None \
                    and not str(ins.queue).startswith("qPoolDynamic"):
                continue
            if tn == "InstDMACopy" and getattr(ins, "queue", None) is None:
                continue
            si = ins.sync_info
            if not si or not si.on_update:
                continue
            m = pat.match(si.on_update[0].ant_name or "")
            if not m:
                continue
            q = int(m.group(1)) % nq
            if tn == "InstDMAGatherAnt":
                ins.queue_num = q
            else:
                ins.queue = f"qPoolDynamic{q if q else ''}"


# ---------------- public entry point ----------------
def build_for_sim(user_weight, item_weight, edge_vals, edge_row, edge_col,
                  user_index, pos_index, neg_index, num_layers, _debug=False):
    """Build the compiled module + per-core input maps without executing."""
    return _prepare(user_weight, item_weight, edge_vals, edge_row, edge_col,
                    user_index, pos_index, neg_index, num_layers, _debug)


def _prepare(user_weight, item_weight, edge_vals, edge_row, edge_col,
             user_index, pos_index, neg_index, num_layers, _debug=False):
    user_weight = np.asarray(user_weight, dtype=np.float32)
    item_weight = np.asarray(item_weight, dtype=np.float32)
    edge_vals = np.asarray(edge_vals, dtype=np.float32)
    edge_row = np.asarray(edge_row, dtype=np.int64)
    edge_col = np.asarray(edge_col, dtype=np.int64)
    user_index = np.asarray(user_index, dtype=np.int64)
    pos_index = np.asarray(pos_index, dtype=np.int64)
    neg_index = np.asarray(neg_index, dtype=np.int64)
    L = int(num_layers)

    pre = _preprocess(edge_row, edge_col, edge_vals)
    core_n, loc_n = pre["core"], pre["loc"]

    x0_nodes = np.concatenate([user_weight, item_weight], axis=0)
    x0_shards = np.zeros((NCORES, SHARD, DIM), np.float32)
    x0_shards[core_n, loc_n] = x0_nodes

    iota = np.tile(np.arange(P, dtype=np.float32),
                   (P, 1)).astype(ml_dtypes.bfloat16)

    # BPR pooled-side compaction: per-core unique local rows referenced
    node_u = user_index
    node_p = NUM_USERS + pos_index
    node_n = NUM_USERS + neg_index
    trip_nodes = {"u": node_u, "p": node_p, "n": node_n}
    allk = np.unique(np.concatenate(
        [core_n[v] * SHARD + loc_n[v] for v in trip_nodes.values()]))
    core_of = allk // SHARD
    within = allk % SHARD
    uniq0 = [within[(core_of == c) & (within < REG_BASE[2])]
             for c in range(NCORES)]
    uniq1 = [within[(core_of == c) & (within >= REG_BASE[2])]
             for c in range(NCORES)]
    S3A = max(128, -(-max(len(x) for x in uniq0) // 128) * 128)
    S3B = max(128, -(-max(len(x) for x in uniq1) // 128) * 128)
    assert NCORES * S3A + NCORES * (S3B + 1) < 32768, (S3A, S3B)
    comp_idx_arr = []
    slot_map = np.zeros(NCORES * SHARD, dtype=np.int64)
    for c in range(NCORES):
        pad = np.zeros(S3A + S3B, np.int64)
        pad[:len(uniq0[c])] = uniq0[c]
        pad[S3A:S3A + len(uniq1[c])] = uniq1[c]
        comp_idx_arr.append(_idx16_layout(pad.astype(np.int16)))
        slot_map[c * SHARD + uniq0[c]] = c * S3A + np.arange(len(uniq0[c]))
        slot_map[c * SHARD + uniq1[c]] = (NCORES * S3A + c * (S3B + 1)
                                          + np.arange(len(uniq1[c])))
    b_tok = {k: _idx16_layout(
        slot_map[core_n[v] * SHARD + loc_n[v]].astype(np.int16))
        for k, v in trip_nodes.items()}

    # L2-reg multiplicities: count of each local row among the 3*BATCH samples
    all_nodes = np.concatenate([node_u, node_p, node_n])
    M = np.zeros((NCORES, SHARD), np.float32)
    np.add.at(M, (core_n[all_nodes], loc_n[all_nodes]), 1.0)
    reg_mult_arr = [M[c].reshape(NDW, P).T.copy() for c in range(NCORES)]
    reg_slot_list = np.array(
        [NCORES * S3A + c * (S3B + 1) + S3B for c in range(NCORES)], np.int64)
    reg_slots_arr = _idx16_layout(
        np.tile(reg_slot_list, P // NCORES).astype(np.int16))

    nc = _build_kernel(L, pre["sched"], pre["toktot"], S3A, S3B,
                       debug_outputs=_debug)

    in_maps = []
    for c in range(NCORES):
        in_maps.append({
            "x0_shard": x0_shards[c],
            "tok_idx": _idx16_layout(pre["sloc"][c]),
            "tok_val": _pm_layout(pre["val"][c]),
            "tok_dloc": _pm_layout(pre["dloc"][c]),
            "iota_in": iota,
            "bpr_u": b_tok["u"], "bpr_p": b_tok["p"], "bpr_n": b_tok["n"],
            "comp_idx": comp_idx_arr[c],
            "reg_mult": reg_mult_arr[c],
            "reg_slots": reg_slots_arr,
        })
    _prepare.last_maps = (core_n, loc_n)
    return nc, in_maps


def kernel(user_weight, item_weight, edge_vals, edge_row, edge_col,
           user_index, pos_index, neg_index, num_layers, _debug=False):
    nc, in_maps = _prepare(user_weight, item_weight, edge_vals, edge_row,
                           edge_col, user_index, pos_index, neg_index,
                           num_layers, _debug)
    from concourse.bass_utils import run_bass_kernel_spmd
    kernel._cache = (nc, in_maps)
    res = run_bass_kernel_spmd(nc, in_maps, core_ids=list(range(NCORES)))
    out = res.results[0]["out_loss"]
    loss1 = np.float32(out[0, 0])
    reg = np.float32(out[0, 1])
    if _debug:
        pooled = np.stack(
            [res.results[c]["pooled_shard_out"] for c in range(NCORES)], axis=0)
        kernel._debug_pooled = (pooled, _prepare.last_maps)
    return loss1, reg
